# revision 6
# baseline (speedup 1.0000x reference)
"""Trainium2 Bass kernel for nn_Decoder (pairwise-grid MLP decoder).

Math (reference):
  out_notes = zV @ zW.T + bu @ bp.T                        (s1, s2)
  pairs[i,j] = [zV[i], zW[j]]                              (s1*s2, 2d)
  theta = relu(pairs @ W1 + b1) @ W2 + b2                  (s1*s2, T)
  p     = softmax(theta, -1)
  hv    = p @ W3 + b3                                      (s1*s2, V)
  hbn   = batchnorm over rows (biased var, eps=1e-5)
  out_text = softmax(hbn, -1)

Kernel strategy (8 cores, shard s1):
  * pairs @ W1 factorizes: zV_i @ W1[:d] + zW @ W1[d:] -> never materialize pairs.
  * BatchNorm stats over ALL rows reduce to the Gram matrix G = p_aug^T p_aug
    (p_aug = [p, ones], 151x151) because hv = p @ W3:
       sum_r hv       = colsum_p @ W3
       sum_r hv^2 (v) = sum_jk G[j,k] W3[j,v] W3[k,v] = colsum(W3 o (G @ W3))
    Only G (91KB) is AllReduce'd across cores.
  * BN folds into the last matmul: W3' = W3 * rstd (columns), plus bias row
    c = -mean*rstd carried by the ones-row of p_aug^T. Final softmax is a
    single fused exp+rowsum ACT pass + one DVE scale pass.
  * b3 cancels out of out_text (BN subtracts it back off), so it is unused.
"""

import numpy as np

import concourse.bass as bass
import concourse.mybir as mybir
import concourse.tile as tile
from concourse import bacc
from concourse.masks import make_identity

NCORES = 8
S1, S2, D, H, T, V = 64, 1024, 50, 80, 150, 2000
RPC = S1 // NCORES          # zV rows per core (8)
ROWS = RPC * S2             # pair rows per core (8192)
NCHUNK = ROWS // 128        # 64
TAUG = T + 1                # 151: p columns + ones column
KB = T - 128                # 22: second contraction chunk (classes 128..149)
KBA = KB + 1                # 23: + ones/c row
VCH = 4                     # v chunks for PSUM (N<=512)
VCS = V // VCH              # 500
NTOT = S1 * S2              # 65536
BN_EPS = 1e-5

F32 = mybir.dt.float32
AF = mybir.ActivationFunctionType


def build_nc():
    nc = bacc.Bacc(
        "TRN2", target_bir_lowering=False, debug=False, num_devices=NCORES
    )
    zVTa = nc.dram_tensor("zVTa", [D + 1, RPC], F32, kind="ExternalInput")
    zWTa = nc.dram_tensor("zWTa", [D + 1, S2], F32, kind="ExternalInput")
    W1a = nc.dram_tensor("W1a", [D, H], F32, kind="ExternalInput")
    W1b = nc.dram_tensor("W1b", [D, H], F32, kind="ExternalInput")
    b1c = nc.dram_tensor("b1c", [H, 1], F32, kind="ExternalInput")
    W2 = nc.dram_tensor("W2", [H, T], F32, kind="ExternalInput")
    b2r = nc.dram_tensor("b2r", [1, T], F32, kind="ExternalInput")
    W3a = nc.dram_tensor("W3a", [128, V], F32, kind="ExternalInput")
    W3b = nc.dram_tensor("W3b", [KB, V], F32, kind="ExternalInput")
    notes = nc.dram_tensor("notes", [RPC, S2], F32, kind="ExternalOutput")
    otext = nc.dram_tensor("otext", [ROWS, V], F32, kind="ExternalOutput")

    with tile.TileContext(nc) as tc:
        with (
            tc.tile_pool(name="const", bufs=1) as const,
            tc.tile_pool(name="dram", bufs=1, space="DRAM") as dram,
        ):
            ident = const.tile([128, 128], F32)
            make_identity(nc, ident[:])
            ones_s = const.tile([128, 1], F32)
            nc.vector.memset(ones_s[:], 1.0)
            ones_r = const.tile([1, 128], F32)
            nc.vector.memset(ones_r[:], 1.0)

            zVTa_s = const.tile([D + 1, RPC], F32)
            nc.sync.dma_start(out=zVTa_s[:], in_=zVTa.ap())
            zWTa_s = const.tile([D + 1, S2], F32)
            nc.sync.dma_start(out=zWTa_s[:], in_=zWTa.ap())
            W1a_s = const.tile([D, H], F32)
            nc.sync.dma_start(out=W1a_s[:], in_=W1a.ap())
            W1b_s = const.tile([D, H], F32)
            nc.sync.dma_start(out=W1b_s[:], in_=W1b.ap())
            b1c_s = const.tile([H, 1], F32)
            nc.sync.dma_start(out=b1c_s[:], in_=b1c.ap())
            W2_s = const.tile([H, T], F32)
            nc.sync.dma_start(out=W2_s[:], in_=W2.ap())
            b2_s = const.tile([1, T], F32)
            nc.sync.dma_start(out=b2_s[:], in_=b2r.ap())
            W3a_s = const.tile([128, V], F32)
            nc.sync.dma_start(out=W3a_s[:], in_=W3a.ap())
            W3b_s = const.tile([KB, V], F32)
            nc.sync.dma_start(out=W3b_s[:], in_=W3b.ap())

            # persistent phase-1 products
            pTa = const.tile([128, ROWS], F32)   # p^T classes 0..127
            pTb = const.tile([KBA, ROWS], F32)   # p^T classes 128..149 + ones row
            AT = const.tile([H, S2], F32)        # (zW @ W1b)^T
            biasT = const.tile([H, RPC], F32)    # (zV_blk @ W1a)^T + b1

            g_in = dram.tile([TAUG, TAUG], F32)
            g_out = dram.tile([TAUG, TAUG], F32)
            rstd_d = dram.tile([1, V], F32)
            w3x_d = dram.tile([KBA, V], F32)     # rows 0..21: W3[128:150]; row 22: -mean

            # ---- out_notes: [zV|bu] @ [zW|bp]^T ----
            with (
                tc.tile_pool(name="nps", bufs=1, space="PSUM") as npp,
                tc.tile_pool(name="nsb", bufs=1) as nsb,
            ):
                pn = npp.tile([RPC, S2], F32)
                nc.tensor.matmul(
                    pn[:, 0:512], zVTa_s[:], zWTa_s[:, 0:512], start=True, stop=True
                )
                nc.tensor.matmul(
                    pn[:, 512:1024], zVTa_s[:], zWTa_s[:, 512:1024],
                    start=True, stop=True,
                )
                sn = nsb.tile([RPC, S2], F32)
                nc.vector.tensor_copy(out=sn[:], in_=pn[:])
                nc.sync.dma_start(out=notes.ap(), in_=sn[:])

                # ---- AT, biasT (reuse pools) ----
                pat = npp.tile([H, S2], F32, tag="pat")
                nc.tensor.matmul(
                    pat[:, 0:512], W1b_s[:], zWTa_s[0:D, 0:512], start=True, stop=True
                )
                nc.tensor.matmul(
                    pat[:, 512:1024], W1b_s[:], zWTa_s[0:D, 512:1024],
                    start=True, stop=True,
                )
                nc.vector.tensor_copy(out=AT[:], in_=pat[:])
                pbt = npp.tile([H, RPC], F32, tag="pbt")
                nc.tensor.matmul(
                    pbt[:], W1a_s[:], zVTa_s[0:D, :], start=True, stop=True
                )
                nc.vector.tensor_scalar_add(biasT[:], pbt[:], b1c_s[:])

            # ---- phase 1: p per chunk, G accumulation, p^T slabs ----
            with (
                tc.tile_pool(name="p1p", bufs=2, space="PSUM") as p1p,
                tc.tile_pool(name="gps", bufs=1, space="PSUM") as gpp,
                tc.tile_pool(name="p1s", bufs=3) as p1s,
                tc.tile_pool(name="hts", bufs=2) as hts,
            ):
                Ga = gpp.tile([128, TAUG], F32)
                Gb = gpp.tile([KBA, TAUG], F32)
                for i in range(RPC):
                    hT = hts.tile([H, S2], F32, tag="hT")
                    nc.scalar.activation(
                        hT[:], AT[:], AF.Relu, bias=biasT[:, i : i + 1],
                        scale=1.0,
                    )
                    for jb in range(8):
                        k = i * 8 + jb
                        th = p1p.tile([128, T], F32, tag="th")
                        nc.tensor.matmul(
                            th[:], hT[:, jb * 128 : (jb + 1) * 128], W2_s[:],
                            start=True, stop=False,
                        )
                        nc.tensor.matmul(
                            th[:], ones_r[:], b2_s[:], start=False, stop=True
                        )
                        e = p1s.tile([128, TAUG], F32, tag="e")
                        nc.scalar.activation(
                            e[:, 0:T], th[:], AF.Exp, accum_out=e[:, T : T + 1]
                        )
                        rcp = p1s.tile([128, 1], F32, tag="rcp")
                        nc.vector.reciprocal(rcp[:], e[:, T : T + 1])
                        pa = p1s.tile([128, TAUG], F32, tag="pa")
                        nc.vector.tensor_scalar_mul(pa[:], e[:], rcp[:])
                        nc.tensor.matmul(
                            Ga[:], pa[:, 0:128], pa[:],
                            start=(k == 0), stop=(k == NCHUNK - 1),
                        )
                        nc.tensor.matmul(
                            Gb[:], pa[:, 128:TAUG], pa[:],
                            start=(k == 0), stop=(k == NCHUNK - 1),
                        )
                        ta = p1p.tile([128, 128], F32, tag="ta")
                        nc.tensor.transpose(ta[:], pa[:, 0:128], ident[:])
                        tb = p1p.tile([KBA, 128], F32, tag="tb")
                        nc.tensor.transpose(tb[:], pa[:, 128:TAUG], ident[:])
                        nc.vector.tensor_copy(
                            out=pTa[:, k * 128 : (k + 1) * 128], in_=ta[:]
                        )
                        nc.vector.tensor_copy(
                            out=pTb[:, k * 128 : (k + 1) * 128], in_=tb[:]
                        )
                gsa = p1s.tile([128, TAUG], F32, tag="gsa")
                nc.vector.tensor_copy(out=gsa[:], in_=Ga[:])
                nc.gpsimd.dma_start(out=g_in[0:128, :], in_=gsa[:])
                gsb = p1s.tile([KBA, TAUG], F32, tag="gsb")
                nc.vector.tensor_copy(out=gsb[:], in_=Gb[:])
                nc.gpsimd.dma_start(out=g_in[128:TAUG, :], in_=gsb[:])

            nc.gpsimd.collective_compute(
                "AllReduce",
                mybir.AluOpType.add,
                replica_groups=[list(range(NCORES))],
                ins=[g_in.opt()],
                outs=[g_out.opt()],
            )

            # ---- stats from reduced G ----
            G2a = const.tile([128, TAUG], F32)
            nc.sync.dma_start(out=G2a[:], in_=g_out[0:128, :])
            G2b = const.tile([KBA, TAUG], F32)
            nc.sync.dma_start(out=G2b[:], in_=g_out[128:TAUG, :])
            W3ap = const.tile([128, V], F32)    # W3 * rstd, rows 0..127
            W3bp = const.tile([KBA, V], F32)    # rows 128..149 scaled + c row
            with (
                tc.tile_pool(name="stp", bufs=2, space="PSUM") as stp,
                tc.tile_pool(name="sts", bufs=1) as sts,
            ):
                meanS = sts.tile([1, V], F32, tag="meanS")
                S2r = sts.tile([1, V], F32, tag="S2r")
                for c in range(VCH):
                    sl = slice(c * VCS, (c + 1) * VCS)
                    MA = stp.tile([128, VCS], F32, tag="MA")
                    nc.tensor.matmul(
                        MA[:], G2a[:, 0:128], W3a_s[:, sl], start=True, stop=False
                    )
                    nc.tensor.matmul(
                        MA[:], G2b[0:KB, 0:128], W3b_s[:, sl], start=False, stop=True
                    )
                    MB = stp.tile([KB, VCS], F32, tag="MB")
                    nc.tensor.matmul(
                        MB[:], G2a[:, 128:T], W3a_s[:, sl],
                        start=True, stop=False,
                    )
                    nc.tensor.matmul(
                        MB[:], G2b[0:KB, 128:T], W3b_s[:, sl],
                        start=False, stop=True,
                    )
                    mS = stp.tile([1, VCS], F32, tag="mS")
                    nc.tensor.matmul(
                        mS[:], G2a[:, T:TAUG], W3a_s[:, sl], start=True, stop=False
                    )
                    nc.tensor.matmul(
                        mS[:], G2b[0:KB, T:TAUG], W3b_s[:, sl],
                        start=False, stop=True,
                    )
                    nc.vector.tensor_copy(out=meanS[:, sl], in_=mS[:])
                    wma = sts.tile([128, VCS], F32, tag="wma")
                    nc.vector.tensor_mul(wma[:], W3a_s[:, sl], MA[:])
                    wmb = sts.tile([KB, VCS], F32, tag="wmb")
                    nc.vector.tensor_mul(wmb[:], W3b_s[:, sl], MB[:])
                    S2p = stp.tile([1, VCS], F32, tag="S2p")
                    nc.tensor.matmul(
                        S2p[:], ones_s[:], wma[:], start=True, stop=False
                    )
                    nc.tensor.matmul(
                        S2p[:], ones_s[0:KB, :], wmb[:], start=False, stop=True
                    )
                    nc.vector.tensor_copy(out=S2r[:, sl], in_=S2p[:])

                mn = sts.tile([1, V], F32, tag="mn")
                nc.vector.tensor_scalar_mul(mn[:], meanS[:], 1.0 / NTOT)
                vr = sts.tile([1, V], F32, tag="vr")
                nc.vector.tensor_scalar_mul(vr[:], S2r[:], 1.0 / NTOT)
                mn2 = sts.tile([1, V], F32, tag="mn2")
                nc.vector.tensor_mul(mn2[:], mn[:], mn[:])
                nc.vector.tensor_sub(vr[:], vr[:], mn2[:])
                eps_t = sts.tile([1, 1], F32, tag="eps")
                nc.vector.memset(eps_t[:], BN_EPS)
                sq = sts.tile([1, V], F32, tag="sq")
                nc.scalar.activation(
                    sq[:], vr[:], AF.Sqrt, bias=eps_t[:], scale=1.0
                )
                rstd = sts.tile([1, V], F32, tag="rstd")
                nc.vector.reciprocal(rstd[:], sq[:])
                mneg = sts.tile([1, V], F32, tag="mneg")
                nc.vector.tensor_scalar_mul(mneg[:], mn[:], -1.0)
                nc.sync.dma_start(out=rstd_d[:], in_=rstd[:])

                # assemble [W3[128:150]; -mean] in DRAM, reload as one tile
                nc.sync.dma_start(out=w3x_d[0:KB, :], in_=W3b_s[:])
                nc.sync.dma_start(out=w3x_d[KB:KBA, :], in_=mneg[:])
                w3x_s = sts.tile([KBA, V], F32, tag="w3x")
                nc.sync.dma_start(out=w3x_s[:], in_=w3x_d[:])

                # broadcast rstd across partitions; scale columns
                rflat = rstd_d[:].rearrange("a b -> (a b)")
                rb = sts.tile([128, V], F32, tag="rb")
                nc.sync.dma_start(
                    out=rb[:],
                    in_=bass.AP(
                        tensor=rflat.tensor, offset=rflat.offset,
                        ap=[[0, 128]] + rflat.ap,
                    ),
                )
                nc.vector.tensor_mul(W3ap[:], W3a_s[:], rb[:])
                nc.vector.tensor_mul(W3bp[:], w3x_s[:], rb[0:KBA, :])

            # ---- main loop: z = p_aug^T-slices @ W3'_aug; softmax rows ----
            with (
                tc.tile_pool(name="mpp", bufs=2, space="PSUM") as mpp,
                tc.tile_pool(name="msb", bufs=3) as msb,
            ):
                vsl = [slice(0, 512), slice(512, 1024), slice(1024, 1536),
                       slice(1536, V)]
                for tch in range(NCHUNK):
                    tsl = slice(tch * 128, (tch + 1) * 128)
                    z = mpp.tile([128, V], F32, tag="z")
                    for sl in vsl:
                        nc.tensor.matmul(
                            z[:, sl], pTa[:, tsl], W3ap[:, sl],
                            start=True, stop=False,
                        )
                    for sl in vsl:
                        nc.tensor.matmul(
                            z[:, sl], pTb[:, tsl], W3bp[:, sl],
                            start=False, stop=True,
                        )
                    e2 = msb.tile([128, V], F32, tag="e2")
                    s2t = msb.tile([128, 1], F32, tag="s2t")
                    nc.scalar.activation(e2[:], z[:], AF.Exp, accum_out=s2t[:])
                    r2 = msb.tile([128, 1], F32, tag="r2")
                    nc.vector.reciprocal(r2[:], s2t[:])
                    o = msb.tile([128, V], F32, tag="o")
                    nc.vector.tensor_scalar_mul(o[:], e2[:], r2[:])
                    nc.sync.dma_start(out=otext.ap()[tsl, :], in_=o[:])

    nc.compile()
    return nc


def make_in_maps(zV, zW, bu, bp, W1, b1, W2, b2, W3, b3):
    f = lambda x: np.ascontiguousarray(np.asarray(x), dtype=np.float32)
    zV, zW, bu, bp = f(zV), f(zW), f(bu), f(bp)
    W1, b1, W2, b2, W3 = f(W1), f(b1), f(W2), f(b2), f(W3)
    zWTa = np.ascontiguousarray(np.concatenate([zW.T, bp.T], axis=0))
    W1a = np.ascontiguousarray(W1[:D])
    W1b = np.ascontiguousarray(W1[D:])
    b1c = np.ascontiguousarray(b1.reshape(H, 1))
    b2r = np.ascontiguousarray(b2.reshape(1, T))
    W3a = np.ascontiguousarray(W3[:128])
    W3b = np.ascontiguousarray(W3[128:T])
    in_maps = []
    for c in range(NCORES):
        zVb = zV[c * RPC : (c + 1) * RPC]
        bub = bu[c * RPC : (c + 1) * RPC]
        zVTa = np.ascontiguousarray(np.concatenate([zVb.T, bub.T], axis=0))
        in_maps.append(
            dict(
                zVTa=zVTa, zWTa=zWTa, W1a=W1a, W1b=W1b, b1c=b1c,
                W2=W2, b2r=b2r, W3a=W3a, W3b=W3b,
            )
        )
    return in_maps


_NC = None


def _get_nc():
    global _NC
    if _NC is None:
        _NC = build_nc()
    return _NC


def kernel(zV, zW, bu, bp, W1, b1, W2, b2, W3, b3, _run=None):
    from concourse.bass_utils import run_bass_kernel_spmd

    nc = _get_nc()
    in_maps = make_in_maps(zV, zW, bu, bp, W1, b1, W2, b2, W3, b3)
    if _run is None:
        res = run_bass_kernel_spmd(nc, in_maps, core_ids=list(range(NCORES)))
        results = res.results
    else:
        results = _run(nc, in_maps)
    out_notes = np.concatenate([results[i]["notes"] for i in range(NCORES)], 0)
    out_text = np.concatenate([results[i]["otext"] for i in range(NCORES)], 0)
    return out_notes, out_text


# revision 13
# speedup vs baseline: 1.9288x; 1.9288x over previous
"""Trainium2 Bass kernel for nn_Decoder (pairwise-grid MLP decoder).

Math (reference):
  out_notes = zV @ zW.T + bu @ bp.T                        (s1, s2)
  pairs[i,j] = [zV[i], zW[j]]                              (s1*s2, 2d)
  theta = relu(pairs @ W1 + b1) @ W2 + b2                  (s1*s2, T)
  p     = softmax(theta, -1)
  hv    = p @ W3 + b3                                      (s1*s2, V)
  hbn   = batchnorm over rows (biased var, eps=1e-5)
  out_text = softmax(hbn, -1)

Kernel strategy (8 cores, shard s1):
  * pairs @ W1 factorizes: zV_i @ W1[:d] + zW @ W1[d:] -> never materialize pairs.
  * BatchNorm stats over ALL rows reduce to the Gram matrix G = p_aug^T p_aug
    (p_aug = [p, ones], 151x151) because hv = p @ W3:
       sum_r hv       = colsum_p @ W3
       sum_r hv^2 (v) = sum_jk G[j,k] W3[j,v] W3[k,v] = colsum(W3 o (G @ W3))
    Only G (91KB) is AllReduce'd across cores.
  * BN folds into the last matmul: W3' = W3 * rstd (columns), plus bias row
    c = -mean*rstd carried by the ones-row of p_aug^T. Final softmax is a
    single fused exp+rowsum ACT pass + one DVE scale pass.
  * b3 cancels out of out_text (BN subtracts it back off), so it is unused.
"""

import numpy as np

import concourse.bass as bass
import concourse.mybir as mybir
import concourse.tile as tile
from concourse import bacc
from concourse.masks import make_identity

NCORES = 8
S1, S2, D, H, T, V = 64, 1024, 50, 80, 150, 2000
RPC = S1 // NCORES          # zV rows per core (8)
ROWS = RPC * S2             # pair rows per core (8192)
NCHUNK = ROWS // 128        # 64
TAUG = T + 1                # 151: p columns + ones column
KB = T - 128                # 22: second contraction chunk (classes 128..149)
KBA = KB + 1                # 23: + ones/c row
VCH = 4                     # v chunks for PSUM (N<=512)
VCS = V // VCH              # 500
NTOT = S1 * S2              # 65536
BN_EPS = 1e-5

F32 = mybir.dt.float32
F32R = mybir.dt.float32r
AF = mybir.ActivationFunctionType


def build_nc():
    nc = bacc.Bacc(
        "TRN2", target_bir_lowering=False, debug=False, num_devices=NCORES
    )
    zVTa = nc.dram_tensor("zVTa", [D + 1, RPC], F32, kind="ExternalInput")
    zWTa = nc.dram_tensor("zWTa", [D + 1, S2], F32, kind="ExternalInput")
    W1a = nc.dram_tensor("W1a", [D, H], F32, kind="ExternalInput")
    W1b = nc.dram_tensor("W1b", [D, H], F32, kind="ExternalInput")
    b1c = nc.dram_tensor("b1c", [H, 1], F32, kind="ExternalInput")
    HA = 97  # W2 rows: 80 real + 16 zeros (quad pad) + b2 row at 96
    W2 = nc.dram_tensor("W2", [HA, T], F32, kind="ExternalInput")
    W3a = nc.dram_tensor("W3a", [128, V], F32, kind="ExternalInput")
    W3b = nc.dram_tensor("W3b", [KB, V], F32, kind="ExternalInput")
    notes = nc.dram_tensor("notes", [RPC, S2], F32, kind="ExternalOutput")
    otext = nc.dram_tensor("otext", [ROWS, V], F32, kind="ExternalOutput")

    with tile.TileContext(nc) as tc:
        with (
            tc.tile_pool(name="const", bufs=1) as const,
            tc.tile_pool(name="dram", bufs=1, space="DRAM") as dram,
        ):
            ident = const.tile([128, 128], F32)
            make_identity(nc, ident[:])
            ones_s = const.tile([128, 1], F32)
            nc.vector.memset(ones_s[:], 1.0)
            # hT: rows 0..79 = relu(...), 80..95 = 0 (quad pad), 96 = 1 (b2 row)
            hT = const.tile([HA, S2], F32)
            nc.vector.memset(hT[64:HA, :], 0.0)
            nc.vector.memset(hT[96:HA, :], 1.0)

            zVTa_s = const.tile([D + 1, RPC], F32)
            nc.sync.dma_start(out=zVTa_s[:], in_=zVTa.ap())
            zWTa_s = const.tile([D + 1, S2], F32)
            nc.sync.dma_start(out=zWTa_s[:], in_=zWTa.ap())
            W1a_s = const.tile([D, H], F32)
            nc.sync.dma_start(out=W1a_s[:], in_=W1a.ap())
            W1b_s = const.tile([D, H], F32)
            nc.sync.dma_start(out=W1b_s[:], in_=W1b.ap())
            b1c_s = const.tile([H, 1], F32)
            nc.sync.dma_start(out=b1c_s[:], in_=b1c.ap())
            W2_s = const.tile([HA, T], F32)
            nc.sync.dma_start(out=W2_s[:], in_=W2.ap())
            W3a_s = const.tile([128, V], F32)
            nc.sync.dma_start(out=W3a_s[:], in_=W3a.ap())
            W3b_s = const.tile([KB, V], F32)
            nc.sync.dma_start(out=W3b_s[:], in_=W3b.ap())

            # persistent phase-1 products
            pTa = const.tile([128, ROWS], F32R)  # p^T classes 0..127 (tf32)
            pTb = const.tile([KBA, ROWS], F32R)  # p^T classes 128..149 + ones row
            AT = const.tile([H, S2], F32)        # (zW @ W1b)^T
            biasT = const.tile([H, RPC], F32)    # (zV_blk @ W1a)^T + b1

            g_in = dram.tile([TAUG, TAUG], F32)
            g_out = dram.tile([TAUG, TAUG], F32)
            rstd_d = dram.tile([1, V], F32)
            w3x_d = dram.tile([KBA, V], F32)     # rows 0..21: W3[128:150]; row 22: -mean

            # ---- out_notes: [zV|bu] @ [zW|bp]^T ----
            with (
                tc.tile_pool(name="nps", bufs=1, space="PSUM") as npp,
                tc.tile_pool(name="nsb", bufs=1) as nsb,
            ):
                pn = npp.tile([RPC, S2], F32)
                nc.tensor.matmul(
                    pn[:, 0:512], zVTa_s[:], zWTa_s[:, 0:512], start=True, stop=True
                )
                nc.tensor.matmul(
                    pn[:, 512:1024], zVTa_s[:], zWTa_s[:, 512:1024],
                    start=True, stop=True,
                )
                sn = nsb.tile([RPC, S2], F32)
                nc.vector.tensor_copy(out=sn[:], in_=pn[:])
                nc.sync.dma_start(out=notes.ap(), in_=sn[:])

                # ---- AT, biasT (reuse pools) ----
                pat = npp.tile([H, S2], F32, tag="pat")
                nc.tensor.matmul(
                    pat[:, 0:512], W1b_s[:], zWTa_s[0:D, 0:512], start=True, stop=True
                )
                nc.tensor.matmul(
                    pat[:, 512:1024], W1b_s[:], zWTa_s[0:D, 512:1024],
                    start=True, stop=True,
                )
                nc.vector.tensor_copy(out=AT[:], in_=pat[:])
                pbt = npp.tile([H, RPC], F32, tag="pbt")
                nc.tensor.matmul(
                    pbt[:], W1a_s[:], zVTa_s[0:D, :], start=True, stop=True
                )
                nc.vector.tensor_scalar_add(biasT[:], pbt[:], b1c_s[:])

            # ---- phase 1: p per chunk, G accumulation, p^T slabs ----
            with (
                tc.tile_pool(name="p1p", bufs=2, space="PSUM") as p1p,
                tc.tile_pool(name="gps", bufs=1, space="PSUM") as gpp,
                tc.tile_pool(name="p1s", bufs=3) as p1s,
                tc.tile_pool(name="hts", bufs=2) as hts,
            ):
                Ga = gpp.tile([128, TAUG], F32)
                Gb = gpp.tile([KBA, TAUG], F32)
                for i in range(RPC):
                    nc.scalar.activation(
                        hT[0:H, :], AT[:], AF.Relu, bias=biasT[:, i : i + 1],
                        scale=1.0,
                    )
                    for jb in range(8):
                        k = i * 8 + jb
                        th = p1p.tile([128, T], F32, tag="th")
                        nc.tensor.matmul(
                            th[:], hT[:, jb * 128 : (jb + 1) * 128], W2_s[:],
                            start=True, stop=True,
                        )
                        e = p1s.tile([128, TAUG], F32, tag="e")
                        nc.scalar.activation(
                            e[:, 0:T], th[:], AF.Exp, accum_out=e[:, T : T + 1]
                        )
                        rcp = p1s.tile([128, 1], F32, tag="rcp")
                        nc.vector.reciprocal(rcp[:], e[:, T : T + 1])
                        pa = p1s.tile([128, TAUG], F32, tag="pa")
                        nc.vector.tensor_scalar_mul(pa[:], e[:], rcp[:])
                        nc.tensor.matmul(
                            Ga[:], pa[:, 0:128], pa[:],
                            start=(k == 0), stop=(k == NCHUNK - 1),
                        )
                        nc.tensor.matmul(
                            Gb[:], pa[:, 128:TAUG], pa[:],
                            start=(k == 0), stop=(k == NCHUNK - 1),
                        )
                        ta = p1p.tile([128, 128], F32, tag="ta")
                        nc.tensor.transpose(ta[:], pa[:, 0:128], ident[:])
                        tb = p1p.tile([KBA, 128], F32, tag="tb")
                        nc.tensor.transpose(tb[:], pa[:, 128:TAUG], ident[:])
                        nc.vector.tensor_copy(
                            out=pTa[:, k * 128 : (k + 1) * 128], in_=ta[:]
                        )
                        nc.vector.tensor_copy(
                            out=pTb[:, k * 128 : (k + 1) * 128], in_=tb[:]
                        )
                gsa = p1s.tile([128, TAUG], F32, tag="gsa")
                nc.vector.tensor_copy(out=gsa[:], in_=Ga[:])
                nc.gpsimd.dma_start(out=g_in[0:128, :], in_=gsa[:])
                gsb = p1s.tile([KBA, TAUG], F32, tag="gsb")
                nc.vector.tensor_copy(out=gsb[:], in_=Gb[:])
                nc.gpsimd.dma_start(out=g_in[128:TAUG, :], in_=gsb[:])

            nc.gpsimd.collective_compute(
                "AllReduce",
                mybir.AluOpType.add,
                replica_groups=[list(range(NCORES))],
                ins=[g_in.opt()],
                outs=[g_out.opt()],
            )

            # ---- stats from reduced G ----
            G2a = const.tile([128, TAUG], F32)
            nc.sync.dma_start(out=G2a[:], in_=g_out[0:128, :])
            G2b = const.tile([KBA, TAUG], F32)
            nc.sync.dma_start(out=G2b[:], in_=g_out[128:TAUG, :])
            W3ap = const.tile([128, V], F32R)   # W3 * rstd, rows 0..127 (tf32)
            W3bp = const.tile([KBA, V], F32R)   # rows 128..149 scaled + c row
            with (
                tc.tile_pool(name="stp", bufs=2, space="PSUM") as stp,
                tc.tile_pool(name="sts", bufs=1) as sts,
            ):
                meanS = sts.tile([1, V], F32, tag="meanS")
                S2r = sts.tile([1, V], F32, tag="S2r")
                for c in range(VCH):
                    sl = slice(c * VCS, (c + 1) * VCS)
                    MA = stp.tile([128, VCS], F32, tag="MA")
                    nc.tensor.matmul(
                        MA[:], G2a[:, 0:128], W3a_s[:, sl], start=True, stop=False
                    )
                    nc.tensor.matmul(
                        MA[:], G2b[0:KB, 0:128], W3b_s[:, sl], start=False, stop=True
                    )
                    MB = stp.tile([KB, VCS], F32, tag="MB")
                    nc.tensor.matmul(
                        MB[:], G2a[:, 128:T], W3a_s[:, sl],
                        start=True, stop=False,
                    )
                    nc.tensor.matmul(
                        MB[:], G2b[0:KB, 128:T], W3b_s[:, sl],
                        start=False, stop=True,
                    )
                    mS = stp.tile([1, VCS], F32, tag="mS")
                    nc.tensor.matmul(
                        mS[:], G2a[:, T:TAUG], W3a_s[:, sl], start=True, stop=False
                    )
                    nc.tensor.matmul(
                        mS[:], G2b[0:KB, T:TAUG], W3b_s[:, sl],
                        start=False, stop=True,
                    )
                    nc.vector.tensor_copy(out=meanS[:, sl], in_=mS[:])
                    wma = sts.tile([128, VCS], F32, tag="wma")
                    nc.vector.tensor_mul(wma[:], W3a_s[:, sl], MA[:])
                    wmb = sts.tile([KB, VCS], F32, tag="wmb")
                    nc.vector.tensor_mul(wmb[:], W3b_s[:, sl], MB[:])
                    S2p = stp.tile([1, VCS], F32, tag="S2p")
                    nc.tensor.matmul(
                        S2p[:], ones_s[:], wma[:], start=True, stop=False
                    )
                    nc.tensor.matmul(
                        S2p[:], ones_s[0:KB, :], wmb[:], start=False, stop=True
                    )
                    nc.vector.tensor_copy(out=S2r[:, sl], in_=S2p[:])

                mn = sts.tile([1, V], F32, tag="mn")
                nc.vector.tensor_scalar_mul(mn[:], meanS[:], 1.0 / NTOT)
                vr = sts.tile([1, V], F32, tag="vr")
                nc.vector.tensor_scalar_mul(vr[:], S2r[:], 1.0 / NTOT)
                mn2 = sts.tile([1, V], F32, tag="mn2")
                nc.vector.tensor_mul(mn2[:], mn[:], mn[:])
                nc.vector.tensor_sub(vr[:], vr[:], mn2[:])
                eps_t = sts.tile([1, 1], F32, tag="eps")
                nc.vector.memset(eps_t[:], BN_EPS)
                sq = sts.tile([1, V], F32, tag="sq")
                nc.scalar.activation(
                    sq[:], vr[:], AF.Sqrt, bias=eps_t[:], scale=1.0
                )
                rstd = sts.tile([1, V], F32, tag="rstd")
                nc.vector.reciprocal(rstd[:], sq[:])
                mneg = sts.tile([1, V], F32, tag="mneg")
                nc.vector.tensor_scalar_mul(mneg[:], mn[:], -1.0)
                nc.sync.dma_start(out=rstd_d[:], in_=rstd[:])

                # assemble [W3[128:150]; -mean] in DRAM, reload as one tile
                nc.sync.dma_start(out=w3x_d[0:KB, :], in_=W3b_s[:])
                nc.sync.dma_start(out=w3x_d[KB:KBA, :], in_=mneg[:])
                w3x_s = sts.tile([KBA, V], F32, tag="w3x")
                nc.sync.dma_start(out=w3x_s[:], in_=w3x_d[:])

                # broadcast rstd across partitions; scale columns
                rflat = rstd_d[:].rearrange("a b -> (a b)")
                rb = sts.tile([128, V], F32, tag="rb")
                nc.sync.dma_start(
                    out=rb[:],
                    in_=bass.AP(
                        tensor=rflat.tensor, offset=rflat.offset,
                        ap=[[0, 128]] + rflat.ap,
                    ),
                )
                nc.vector.tensor_mul(W3ap[:], W3a_s[:], rb[:])
                nc.vector.tensor_mul(W3bp[:], w3x_s[:], rb[0:KBA, :])

            # ---- main loop: z = p_aug^T-slices @ W3'_aug; softmax rows ----
            with (
                tc.tile_pool(name="mpp", bufs=2, space="PSUM") as mpp,
                tc.tile_pool(name="msb", bufs=3) as msb,
            ):
                vsl = [slice(0, 512), slice(512, 1024), slice(1024, 1536),
                       slice(1536, V)]
                for tch in range(NCHUNK):
                    tsl = slice(tch * 128, (tch + 1) * 128)
                    z = mpp.tile([128, V], F32, tag="z")
                    for sl in vsl:
                        nc.tensor.matmul(
                            z[:, sl], pTa[:, tsl], W3ap[:, sl],
                            start=True, stop=False,
                        )
                    for sl in vsl:
                        nc.tensor.matmul(
                            z[:, sl], pTb[:, tsl], W3bp[:, sl],
                            start=False, stop=True,
                        )
                    e2 = msb.tile([128, V], F32, tag="e2")
                    s2t = msb.tile([128, 1], F32, tag="s2t")
                    nc.scalar.activation(e2[:], z[:], AF.Exp, accum_out=s2t[:])
                    r2 = msb.tile([128, 1], F32, tag="r2")
                    nc.vector.reciprocal(r2[:], s2t[:])
                    o = msb.tile([128, V], F32, tag="o")
                    nc.vector.tensor_scalar_mul(o[:], e2[:], r2[:])
                    nc.sync.dma_start(out=otext.ap()[tsl, :], in_=o[:])

    nc.compile()
    return nc


def make_in_maps(zV, zW, bu, bp, W1, b1, W2, b2, W3, b3):
    f = lambda x: np.ascontiguousarray(np.asarray(x), dtype=np.float32)
    zV, zW, bu, bp = f(zV), f(zW), f(bu), f(bp)
    W1, b1, W2, b2, W3 = f(W1), f(b1), f(W2), f(b2), f(W3)
    zWTa = np.ascontiguousarray(np.concatenate([zW.T, bp.T], axis=0))
    W1a = np.ascontiguousarray(W1[:D])
    W1b = np.ascontiguousarray(W1[D:])
    b1c = np.ascontiguousarray(b1.reshape(H, 1))
    W2g = np.ascontiguousarray(
        np.concatenate([W2, np.zeros((16, T), np.float32), b2[None, :]], axis=0)
    )
    W3a = np.ascontiguousarray(W3[:128])
    W3b = np.ascontiguousarray(W3[128:T])
    in_maps = []
    for c in range(NCORES):
        zVb = zV[c * RPC : (c + 1) * RPC]
        bub = bu[c * RPC : (c + 1) * RPC]
        zVTa = np.ascontiguousarray(np.concatenate([zVb.T, bub.T], axis=0))
        in_maps.append(
            dict(
                zVTa=zVTa, zWTa=zWTa, W1a=W1a, W1b=W1b, b1c=b1c,
                W2=W2g, W3a=W3a, W3b=W3b,
            )
        )
    return in_maps


_NC = None


def _get_nc():
    global _NC
    if _NC is None:
        _NC = build_nc()
    return _NC


def kernel(zV, zW, bu, bp, W1, b1, W2, b2, W3, b3, _run=None):
    from concourse.bass_utils import run_bass_kernel_spmd

    nc = _get_nc()
    in_maps = make_in_maps(zV, zW, bu, bp, W1, b1, W2, b2, W3, b3)
    if _run is None:
        res = run_bass_kernel_spmd(nc, in_maps, core_ids=list(range(NCORES)))
        results = res.results
    else:
        results = _run(nc, in_maps)
    out_notes = np.concatenate([results[i]["notes"] for i in range(NCORES)], 0)
    out_text = np.concatenate([results[i]["otext"] for i in range(NCORES)], 0)
    return out_notes, out_text
